# revision 1
# baseline (speedup 1.0000x reference)
"""Entropy-gated multi-head attention on 8 Trainium2 NeuronCores.

Sharding: core c = b*4 + g handles batch b (of 2) and head-group g (4 of the
16 heads).  Tokens with gate==0 pass x through untouched and contribute
exactly zero k/v (zero biases), so the device only processes the compacted
active tokens (~half), with the softmax denominator corrected by the count of
inactive tokens: each inactive key contributes exp(0)=1 to the softmax sum
(scores vs. zeroed k are exactly 0) and nothing to the numerator (v=0).

Device math per core (no max-subtraction; scores are O(5) so exp is safe):
  QT = Wq_g^T x^T, KT = Wk_g^T x^T           [256, SA]
  V  = x Wv_g                                 [SA, 256]
  per head h: PT = exp((KT_h^T QT_h)/8)       [SA_k, SA_q]
              OT' = [V_h | 1]^T PT            [65, SA_q] (row 64 = colsum = Z_act)
              r = 1/(Z_act + (S - SA))        broadcast to [64, SA_q] via DMA
              osb_h = OT * r                  (scaled attention out, lhsT form)
  Y(q, :) = sum_h osb_h^T Wo_h                (accumulated in PSUM)
Host sums the 4 per-group partial Y per batch, adds bo, scatters into x.
"""

import math
from contextlib import ExitStack

import numpy as np
import ml_dtypes

import concourse.bass as bass
import concourse.mybir as mybir
from concourse import bacc
import concourse.tile as tile
from concourse.bass_utils import run_bass_kernel_spmd

B, S, D = 2, 2048, 1024
H, DH = 16, 64
NCORES = 8
GROUPS = NCORES // B          # head-groups per batch = 4
HC = H // GROUPS              # heads per core = 4
DC = HC * DH                  # head-group width = 256

# matmul operand dtype: "bf16" | "f32" | "f32r"
MM_DTYPE = "bf16"
# pass explicit tile_position on K=64 score matmuls (row-group packing)
ST_TILE_POS = False

_DT = {
    "bf16": mybir.dt.bfloat16,
    "f32": mybir.dt.float32,
    "f32r": mybir.dt.float32,
}
_NPDT = {
    "bf16": ml_dtypes.bfloat16,
    "f32": np.float32,
    "f32r": np.float32,
}

f32 = mybir.dt.float32


def _chunks(total, step):
    out = []
    o = 0
    while o < total:
        out.append((o, min(step, total - o)))
        o += step
    return out


def _build(SA: int, dtype_tag: str) -> bass.Bass:
    DT = _DT[dtype_tag]

    def mm(ap):
        if dtype_tag == "f32r":
            return ap.bitcast(mybir.dt.float32r)
        return ap

    nkt = D // 128            # 8 contraction tiles for projections
    nst = SA // 128           # token tiles
    qch = _chunks(SA, 512)    # q chunks
    dch = _chunks(D, 512)     # output-dim chunks
    CADD = float(S - SA)      # inactive keys not represented by padding

    nc = bacc.Bacc()
    xT_d = nc.dram_tensor("xT", [D, SA], DT, kind="ExternalInput")
    wq_d = nc.dram_tensor("wq", [D, DC], DT, kind="ExternalInput")
    wk_d = nc.dram_tensor("wk", [D, DC], DT, kind="ExternalInput")
    wv_d = nc.dram_tensor("wv", [D, DC], DT, kind="ExternalInput")
    wo_d = nc.dram_tensor("wo", [DC, D], DT, kind="ExternalInput")
    y_d = nc.dram_tensor("y", [SA, D], f32, kind="ExternalOutput")

    with tile.TileContext(nc) as tc, ExitStack() as ctx:
        singles = ctx.enter_context(tc.tile_pool(name="singles", bufs=1))
        pt_pool = ctx.enter_context(tc.tile_pool(name="pt", bufs=6))
        otsb_pool = ctx.enter_context(tc.tile_pool(name="otsb", bufs=2))
        zr_pool = ctx.enter_context(tc.tile_pool(name="zr", bufs=2))
        rbc_pool = ctx.enter_context(tc.tile_pool(name="rbc", bufs=2))
        yout_pool = ctx.enter_context(tc.tile_pool(name="yout", bufs=3))
        zscr_pool = ctx.enter_context(tc.tile_pool(name="zscr", bufs=4,
                                                   space="DRAM"))
        # PSUM: st pairs 2x2 banks + aux (proj/V/Y) 2x1 + ot 2x1 = 8 banks
        mm_ps = ctx.enter_context(tc.tile_pool(name="mmps", bufs=2, space="PSUM"))
        aux_ps = ctx.enter_context(tc.tile_pool(name="auxps", bufs=2, space="PSUM"))
        ot_ps_pool = ctx.enter_context(tc.tile_pool(name="otps", bufs=2, space="PSUM"))

        # ---- persistent SBUF; DMA order = consumption order for fast start
        wq_sb = singles.tile([128, nkt, DC], DT)
        wk_sb = singles.tile([128, nkt, DC], DT)
        wv_sb = singles.tile([128, nkt, DC], DT)
        xt = singles.tile([128, nkt, SA], DT)
        (c0, c0n) = qch[0]
        for t in range(nkt):
            nc.sync.dma_start(wq_sb[:, t, :], wq_d[t * 128:(t + 1) * 128, :])
            nc.sync.dma_start(wk_sb[:, t, :], wk_d[t * 128:(t + 1) * 128, :])
            nc.sync.dma_start(xt[:, t, c0:c0 + c0n],
                              xT_d[t * 128:(t + 1) * 128, c0:c0 + c0n])
        for (q0, qn) in qch[1:]:
            for t in range(nkt):
                nc.sync.dma_start(xt[:, t, q0:q0 + qn],
                                  xT_d[t * 128:(t + 1) * 128, q0:q0 + qn])
        for t in range(nkt):
            nc.sync.dma_start(wv_sb[:, t, :], wv_d[t * 128:(t + 1) * 128, :])
        wo_sb = []
        for p in range(HC // 2):
            w = singles.tile([128, D], DT, tag=f"wo{p}", name=f"wo{p}")
            nc.sync.dma_start(w, wo_d[p * 128:(p + 1) * 128, :])
            wo_sb.append(w)

        # ---- projections: QT/KT [256, SA], V (augmented with ones) ----
        qt = [singles.tile([128, SA], DT, tag=f"qt{m}", name=f"qt{m}")
              for m in range(2)]
        kt = [singles.tile([128, SA], DT, tag=f"kt{m}", name=f"kt{m}")
              for m in range(2)]
        v_aug = singles.tile([128, nst, HC, 65], DT)

        def proj_qk(m, dst, w_sb, q0, qn):
            ps = aux_ps.tile([128, 512], f32, tag="aux", name="ps")
            for t in range(nkt):
                nc.tensor.matmul(
                    ps[:, :qn],
                    mm(w_sb[:, t, m * 128:(m + 1) * 128]),
                    mm(xt[:, t, q0:q0 + qn]),
                    start=(t == 0), stop=(t == nkt - 1))
            nc.vector.tensor_copy(dst[m][:, q0:q0 + qn], ps[:, :qn])

        v_done = set()

        def proj_v(s):
            v_done.add(s)
            ps = aux_ps.tile([128, 512], f32, tag="aux", name="ps")
            for t in range(nkt):
                nc.tensor.matmul(
                    ps[:, :DC],
                    mm(xt[:, t, s * 128:(s + 1) * 128]),
                    mm(wv_sb[:, t, :]),
                    start=(t == 0), stop=(t == nkt - 1))
            for h in range(HC):
                nc.vector.tensor_copy(v_aug[:, s, h, 0:64],
                                      ps[:, h * 64:(h + 1) * 64])

        # m=0 projections and the first V tiles run up-front (PE-dense, warms
        # HAM); remaining independent PE work (V tail, m=1 projections, Y of
        # finished chunks) is queued and drained between attention groups so
        # the PE never starves while ACT runs the exps.
        aux_jobs = []

        def drain_aux(k):
            for _ in range(min(k, len(aux_jobs))):
                aux_jobs.pop(0)()

        nc.vector.memset(v_aug[:, :, :, 64:65], 1.0)
        for (q0, qn) in qch:
            proj_qk(0, qt, wq_sb, q0, qn)
            proj_qk(0, kt, wk_sb, q0, qn)
        proj_v(0)
        proj_v(1)
        for s in range(2, nst):
            aux_jobs.append(lambda s=s: proj_v(s))
        for (q0, qn) in qch:
            aux_jobs.append(lambda a=q0, b=qn: proj_qk(1, qt, wq_sb, a, b))
            aux_jobs.append(lambda a=q0, b=qn: proj_qk(1, kt, wk_sb, a, b))

        # ---- attention + output projection, per q chunk ----
        for ci, (q0, qn) in enumerate(qch):
            ot_sb = [None] * HC
            for p in range(HC // 2):
                m = p
                if p == 1 and ci == 0:
                    # pair 1 needs the m=1 projections: pull them forward
                    drain_aux(len(aux_jobs))
                ot_ps = {}
                for h in (2 * p, 2 * p + 1):
                    ot_ps[h] = ot_ps_pool.tile([65, 512], f32, tag="ot",
                                               name="ot_ps")
                for si in range(0, nst, 2):
                    # NOTE: emission order is semantic order in Tile — a
                    # consumer emitted before its producer reads stale data.
                    # During chunk0/pair0 the V-projection jobs at the queue
                    # head MUST outpace the OT consumers: 2 jobs per si-group
                    # keeps proj_v(s) strictly ahead of OT reads of v_aug[s].
                    drain_aux(2 if (ci == 0 and p == 0) else 1)
                    npair = min(2, nst - si)
                    assert all(s in v_done for s in range(si, si + npair)), \
                        f"proj_v not emitted before OT consumer: {si}"

                    st, pt = {}, {}
                    for h in (2 * p, 2 * p + 1):
                        st[h] = mm_ps.tile([128, 2, 512], f32, tag="mm",
                                           name="st_ps")
                    # adjacent (even,odd) matmuls with explicit row-group
                    # tile_position run concurrently on the PE array halves
                    for j in range(npair):
                        s = si + j
                        for h in (2 * p, 2 * p + 1):
                            r0 = (h % 2) * 64
                            nc.tensor.matmul(
                                st[h][:, j, :qn],
                                mm(kt[m][r0:r0 + 64, s * 128:(s + 1) * 128]),
                                mm(qt[m][r0:r0 + 64, q0:q0 + qn]),
                                start=True, stop=True,
                                tile_position=(r0, 0))
                    for h in (2 * p, 2 * p + 1):
                        pt[h] = pt_pool.tile([128, 2, 512], DT, tag="pt",
                                             name="pt")
                        nc.scalar.activation(
                            pt[h][:, :npair, :qn], st[h][:, :npair, :qn],
                            mybir.ActivationFunctionType.Exp, scale=0.125)
                    for h in (2 * p, 2 * p + 1):
                        for j in range(npair):
                            s = si + j
                            nc.tensor.matmul(
                                ot_ps[h][:, :qn],
                                mm(v_aug[:, s, h, :]),
                                mm(pt[h][:, j, :qn]),
                                start=(s == 0), stop=(s == nst - 1))
                osbp = otsb_pool.tile([128, 512], DT, tag=f"osbp{p}",
                                      name=f"osbp{p}")
                ot_sb[p] = osbp
                for h in (2 * p, 2 * p + 1):
                    # evacuate psum immediately: unscaled OT (bf16) + Z row;
                    # the r broadcast (DRAM hop) then runs off the critical
                    # path; osb is scaled before the Y matmuls.  Odd heads
                    # are DMA-relocated to partitions 64-127 of the pair tile
                    # so Y can row-pack.
                    zt = zr_pool.tile([65, 512], f32, tag="zt", name="zt")
                    nc.vector.tensor_scalar(
                        out=zt[64:65, :qn], in0=ot_ps[h][64:65, :qn],
                        scalar1=CADD, scalar2=None, op0=mybir.AluOpType.add)
                    zd = zscr_pool.tile([1, 512], f32, tag="zd", name="zd")
                    nc.sync.dma_start(zd[0:1, :qn], zt[64:65, :qn])
                    rb = rbc_pool.tile([64, 512], f32, tag=f"rbc{h}",
                                       name=f"rbc{h}")
                    nc.sync.dma_start(rb[:, :qn],
                                      zd[0:1, :qn].to_broadcast((64, qn)))
                    nc.vector.reciprocal(rb[:, :qn], rb[:, :qn])
                    if h % 2 == 0:
                        nc.vector.scalar_tensor_tensor(
                            out=osbp[0:64, :qn], in0=ot_ps[h][0:64, :qn],
                            scalar=1.0, in1=rb[:, :qn],
                            op0=mybir.AluOpType.mult,
                            op1=mybir.AluOpType.mult)
                    else:
                        otmp = otsb_pool.tile([64, 512], DT, tag="otmp",
                                              name="otmp")
                        nc.vector.scalar_tensor_tensor(
                            out=otmp[:, :qn], in0=ot_ps[h][0:64, :qn],
                            scalar=1.0, in1=rb[:, :qn],
                            op0=mybir.AluOpType.mult,
                            op1=mybir.AluOpType.mult)
                        nc.sync.dma_start(osbp[64:128, :qn], otmp[:, :qn])

            # output projection: row-packed (even,odd) pairs run
            # concurrently into two psum tiles; pairs accumulate; DVE adds.
            # Queued so the Y matmuls fill PE bubbles of the next chunk's
            # (ACT-bound) attention.
            def y_job(q0, qn, jt, osb_pair):
                qtn = min(128, qn - jt * 128)
                for (d0, dn) in dch:
                    yA = aux_ps.tile([128, 512], f32, tag="aux", name="yA")
                    yB = aux_ps.tile([128, 512], f32, tag="aux", name="yB")
                    for p in range(HC // 2):
                        for (yp, r0) in ((yA, 0), (yB, 64)):
                            nc.tensor.matmul(
                                yp[:qtn, :dn],
                                mm(osb_pair[p][r0:r0 + 64,
                                               jt * 128:jt * 128 + qtn]),
                                mm(wo_sb[p][r0:r0 + 64, d0:d0 + dn]),
                                start=(p == 0), stop=(p == HC // 2 - 1),
                                tile_position=(r0, 0))
                    yo = yout_pool.tile([128, 512], f32, tag="yo", name="yo")
                    nc.vector.tensor_copy(yo[:qtn, :dn], yA[:qtn, :dn])
                    nc.vector.tensor_add(yo[:qtn, :dn], yo[:qtn, :dn],
                                         yB[:qtn, :dn])
                    nc.sync.dma_start(
                        y_d[q0 + jt * 128: q0 + jt * 128 + qtn, d0:d0 + dn],
                        yo[:qtn, :dn])

            for jt in range((qn + 127) // 128):
                aux_jobs.append(
                    lambda a=q0, b=qn, j=jt, o=tuple(ot_sb[:HC // 2]):
                    y_job(a, b, j, o))
        drain_aux(len(aux_jobs))
    nc.compile()
    return nc


_nc_cache: dict = {}


def _get_nc(SA: int):
    key = (SA, MM_DTYPE)
    if key not in _nc_cache:
        _nc_cache[key] = _build(SA, MM_DTYPE)
    return _nc_cache[key]


def _reference_fallback(x, gate, Wq, bq, Wk, bk, Wv, bv, Wo, bo):
    g = gate.astype(x.dtype)[..., None]
    q = (x @ Wq + bq) * g
    k = (x @ Wk + bk) * g
    v = (x @ Wv + bv) * g

    def split(t):
        return t.reshape(B, S, H, DH).transpose(0, 2, 1, 3)

    q, k, v = split(q), split(k), split(v)
    sc = np.einsum('bhqd,bhkd->bhqk', q, k) / np.float32(math.sqrt(DH))
    sc = sc - sc.max(axis=-1, keepdims=True)
    e = np.exp(sc)
    attn = e / e.sum(axis=-1, keepdims=True)
    out = np.einsum('bhqk,bhkd->bhqd', attn, v)
    out = out.transpose(0, 2, 1, 3).reshape(B, S, D)
    out = out @ Wo + bo
    return (x * (1.0 - g) + out * g).astype(np.float32)


def kernel(x, gate, Wq, bq, Wk, bk, Wv, bv, Wo, bo, _profile=None):
    x = np.asarray(x, np.float32)
    gate = np.asarray(gate)
    args = dict(x=x, gate=gate, Wq=np.asarray(Wq, np.float32),
                bq=np.asarray(bq, np.float32), Wk=np.asarray(Wk, np.float32),
                bk=np.asarray(bk, np.float32), Wv=np.asarray(Wv, np.float32),
                bv=np.asarray(bv, np.float32), Wo=np.asarray(Wo, np.float32),
                bo=np.asarray(bo, np.float32))

    idxs = [np.nonzero(gate[b])[0] for b in range(B)]
    n_act = [len(i) for i in idxs]
    # the compaction trick needs zero q/k/v biases and at least one active
    # and one inactive token per batch; otherwise fall back to exact numpy
    if (any(np.abs(args[k]).max() > 0 for k in ("bq", "bk", "bv"))
            or min(n_act) == 0 or max(n_act) == S):
        return _reference_fallback(**args)

    SA = ((max(n_act) + 127) // 128) * 128
    npdt = _NPDT[MM_DTYPE]

    in_maps = []
    for b in range(B):
        xa = np.zeros((SA, D), np.float32)
        xa[:n_act[b]] = x[b, idxs[b]]
        xT = np.ascontiguousarray(xa.T).astype(npdt)
        for g in range(GROUPS):
            cs = slice(g * DC, (g + 1) * DC)
            in_maps.append({
                "xT": xT,
                "wq": np.ascontiguousarray(args["Wq"][:, cs]).astype(npdt),
                "wk": np.ascontiguousarray(args["Wk"][:, cs]).astype(npdt),
                "wv": np.ascontiguousarray(args["Wv"][:, cs]).astype(npdt),
                "wo": np.ascontiguousarray(args["Wo"][cs, :]).astype(npdt),
            })

    nc = _get_nc(SA)
    kw = dict(_profile) if _profile else {}
    kw.pop("result", None)
    res = run_bass_kernel_spmd(nc, in_maps, core_ids=list(range(NCORES)), **kw)
    if _profile is not None:
        _profile["result"] = res

    out = x.copy()
    for b in range(B):
        Y = np.zeros((SA, D), np.float32)
        for g in range(GROUPS):
            Y += res.results[b * GROUPS + g]["y"]
        out[b, idxs[b]] = Y[:n_act[b]] + args["bo"]
    return out

